# revision 7
# baseline (speedup 1.0000x reference)
"""Camera back-projection (truncated depth field) Trainium2 kernel.

out[b,0,i,j,k] = relu(1 - 128*|depth[b,0,vi(j,k),ui(i,k)] - zc_k|) with
frustum/validity masking, where (u,v) are pinhole projections of the voxel
grid. 8 cores, 2 batches/core, pure data parallel.

Device pipeline (per batch, per 4-k chunk):
  stage A (PE): psA[r,(k,i)] = sum_c winT[c,r] * Q[c,(k,i)] = d(r,k,i) - zc_k
      winT is the fp16 hi/lo split of the 252^2 depth window (transposed);
      Q is a one-hot fp16 column-selection (ui) with two augmented rows
      carrying -zc_hi/-zc_lo. Exact to ~1e-6.
  tent (ACT/DVE): F[r,(k,i)] = relu(1 - 128*|psA|)  -> fp16 (err <= 2.5e-4)
  stage B (PE): psB[j,(k,i)] = sum_r P[r,(k,j)] * F[r,(k,i)]
      P is a one-hot fp16 row-selection (vi); invalid voxels have all-zero
      one-hot columns and come out exactly 0.
  drain (ACT/DVE): out_sb[j,(k,i)] = psB -> f32 -> contiguous DMA out.
Host: out[b,0,i,j,k] = outdev[b][j,k,i] (pure transpose).
"""
import sys
import numpy as np

sys.path.insert(0, "/opt/trn_rl_repo")

RES = 128
IMG = 480
N = 16
NCORES = 8
BPC = N // NCORES          # batches per core
WIN = 252                  # depth window rows/cols actually used
WPAD = 256                 # padded to 2 partition tiles
KCH = 4                    # k's per pipeline chunk
NCHUNK = RES // KCH
POISON = np.float32(100.0) # fp16-safe "far" depth for invalid samples

_nc_cache = {}


def _build_program():
    import concourse.bacc as bacc
    import concourse.mybir as mybir
    import concourse.tile as tile

    P = 128
    NF = KCH * RES             # free size per chunk (512)
    nc = bacc.Bacc(None, target_bir_lowering=False, debug=False)
    with tile.TileContext(nc) as tc:
        with tc.tile_pool(name="dram", bufs=1, space="DRAM") as dram:
            wts, qs, ps_, outs = {}, {}, {}, {}
            for b in range(BPC):
                for s in ("hi", "lo"):
                    wts[b, s] = dram.tile([2, P, WPAD], mybir.dt.float16,
                                          kind="ExternalInput", uniquify=False, name=f"wt_{s}{b}")
                qs[b] = dram.tile([2, P, RES * RES], mybir.dt.float16,
                                  kind="ExternalInput", uniquify=False, name=f"q{b}")
                ps_[b] = dram.tile([2, P, RES * RES], mybir.dt.float16,
                                   kind="ExternalInput", uniquify=False, name=f"p{b}")
                outs[b] = dram.tile([RES, RES * RES], mybir.dt.float32,
                                    kind="ExternalOutput", uniquify=False, name=f"outdev{b}")

            with (
                tc.tile_pool(name="sb", bufs=1) as sb,
                tc.tile_pool(name="ps", bufs=1, space="PSUM") as ps,
            ):
                for b in range(BPC):
                    wt_sb = {}
                    for s in ("hi", "lo"):
                        for c in range(2):
                            t = sb.tile([P, WPAD], mybir.dt.float16,
                                        name=f"wt_{s}{c}_{b}", tag=f"wt_{s}{c}", bufs=1)
                            nc.sync.dma_start(t[:], wts[b, s][c])
                            wt_sb[s, c] = t

                    for ch in range(NCHUNK):
                        sl = slice(ch * NF, (ch + 1) * NF)
                        qc, pc = {}, {}
                        for c in range(2):
                            t = sb.tile([P, NF], mybir.dt.float16,
                                        name=f"qc{c}_{b}_{ch}", tag=f"qc{c}", bufs=3)
                            nc.sync.dma_start(t[:], qs[b][c, :, sl])
                            qc[c] = t
                        for rt in range(2):
                            t = sb.tile([P, NF], mybir.dt.float16,
                                        name=f"pc{rt}_{b}_{ch}", tag=f"pc{rt}", bufs=3)
                            nc.sync.dma_start(t[:], ps_[b][rt, :, sl])
                            pc[rt] = t

                        psA = {}
                        for rt in range(2):
                            psA[rt] = ps.tile([P, NF], mybir.dt.float32,
                                              name=f"psA{rt}_{b}_{ch}", tag=f"psA{rt}", bufs=2)
                        combos = [("hi", 0), ("hi", 1), ("lo", 0), ("lo", 1)]
                        for m, (s, c) in enumerate(combos):
                            for rt in range(2):
                                nc.tensor.matmul(
                                    psA[rt][:],
                                    wt_sb[s, c][:, rt * P:(rt + 1) * P],
                                    qc[c][:],
                                    start=(m == 0), stop=(m == 3),
                                )

                        # tent -> fp16 F; Abs on ACT (PSUM reader), rest split
                        F = {}
                        for rt in range(2):
                            F[rt] = sb.tile([P, NF], mybir.dt.float16,
                                            name=f"F{rt}_{b}_{ch}", tag=f"F{rt}", bufs=3)
                        a0 = sb.tile([P, NF], mybir.dt.float32,
                                     name=f"a0_{b}_{ch}", tag="a0", bufs=2)
                        nc.scalar.activation(a0[:], psA[0][:],
                                             mybir.ActivationFunctionType.Abs)
                        nc.scalar.activation(F[0][:], a0[:],
                                             mybir.ActivationFunctionType.Relu,
                                             bias=1.0, scale=-128.0)
                        a1 = sb.tile([P, NF], mybir.dt.float32,
                                     name=f"a1_{b}_{ch}", tag="a1", bufs=2)
                        nc.scalar.activation(a1[:], psA[1][:],
                                             mybir.ActivationFunctionType.Abs)
                        t1 = sb.tile([P, NF], mybir.dt.float32,
                                     name=f"t1_{b}_{ch}", tag="t1", bufs=2)
                        nc.vector.tensor_scalar(t1[:], a1[:],
                                                scalar1=-128.0, scalar2=1.0,
                                                op0=mybir.AluOpType.mult,
                                                op1=mybir.AluOpType.add)
                        nc.vector.tensor_scalar(F[1][:], t1[:],
                                                scalar1=0.0, scalar2=None,
                                                op0=mybir.AluOpType.max)

                        # stage B: psB[j, (k,i)] += P_k^T @ F_k per k
                        psB = ps.tile([P, NF], mybir.dt.float32,
                                      name=f"psB_{b}_{ch}", tag="psB", bufs=2)
                        for kc in range(KCH):
                            ksl = slice(kc * RES, (kc + 1) * RES)
                            for rt in range(2):
                                nc.tensor.matmul(
                                    psB[:, ksl],
                                    pc[rt][:, ksl],
                                    F[rt][:, ksl],
                                    start=(rt == 0), stop=(rt == 1),
                                )

                        # drain psB -> f32 out rows, split ACT/DVE
                        ob = sb.tile([P, NF], mybir.dt.float32,
                                     name=f"ob_{b}_{ch}", tag="ob", bufs=3)
                        h = NF // 2
                        nc.scalar.activation(ob[:, :h], psB[:, :h],
                                             mybir.ActivationFunctionType.Copy)
                        nc.vector.tensor_copy(ob[:, h:], psB[:, h:])
                        nc.sync.dma_start(outs[b][:, sl], ob[:])
    nc.compile()
    return nc


def _host_precompute(depth, fl, cd):
    """Per-batch device inputs. Index math in float32, matching the jax
    reference op-for-op."""
    f32 = np.float32
    res = RES
    c = ((np.arange(res, dtype=f32) + f32(0.5)) / f32(res)) - f32(0.5)
    zc = f32(cd) - c                        # [k]
    kvalid = zc > 0
    with np.errstate(divide="ignore", invalid="ignore"):
        u = (f32(fl) * c)[:, None] / zc[None, :] + f32((IMG - 1) * 0.5)  # [i,k] == [j,k]
    ui = np.clip(np.round(u), 0, IMG - 1).astype(np.int64)
    mu = (u >= 0) & (u <= IMG - 1) & kvalid[None, :]

    if mu.any():
        cmin = int(ui[mu].min())
        cmax = int(ui[mu].max())
    else:
        cmin = cmax = 0
    if (cmax - cmin) >= WIN:
        raise NotImplementedError("projection span exceeds window")
    base = min(cmin, IMG - WIN)   # window base for both rows and cols (u==v)

    w = depth[base:base + WIN, base:base + WIN].astype(f32).copy()
    w[w <= 0] = POISON
    wpad = np.zeros((WPAD, WPAD), dtype=f32)
    wpad[:WIN, :WIN] = w
    w_hi = wpad.astype(np.float16)
    w_lo = (wpad - w_hi.astype(f32)).astype(np.float16)
    # winT[c, r] tiles [2, 128, 256]; aug rows at c=254,255 (hi=1.0) carry -zc
    wt_hi = np.ascontiguousarray(w_hi.T).reshape(2, 128, WPAD)
    wt_lo = np.ascontiguousarray(w_lo.T).reshape(2, 128, WPAD)
    wt_hi[1, 126, :] = np.float16(1.0)
    wt_hi[1, 127, :] = np.float16(1.0)
    wt_lo[1, 126:, :] = 0

    nzc = -zc
    nzc_hi = nzc.astype(np.float16)
    nzc_lo = (nzc - nzc_hi.astype(f32)).astype(np.float16)

    # Q[c, (k,i)]: one-hot ui, plus aug rows
    q = np.zeros((2, 128, res * res), dtype=np.float16)
    ii, kk = np.nonzero(mu)
    cloc = (ui[ii, kk] - base).astype(np.int64)
    q[cloc // 128, cloc % 128, kk * res + ii] = np.float16(1.0)
    q[1, 126, :] = np.repeat(np.where(kvalid, nzc_hi, np.float16(0)), res)
    q[1, 127, :] = np.repeat(np.where(kvalid, nzc_lo, np.float16(0)), res)

    # P[r, (k,j)]: one-hot vi (v == u maps with j in place of i)
    p = np.zeros((2, 128, res * res), dtype=np.float16)
    p[cloc // 128, cloc % 128, kk * res + ii] = np.float16(1.0)
    return wt_hi, wt_lo, q, p


def kernel(depth_t, fl, cam_dist):
    from concourse.bass_utils import run_bass_kernel_spmd

    depth_t = np.asarray(depth_t)
    fl = np.asarray(fl).reshape(N)
    cam_dist = np.asarray(cam_dist).reshape(N)

    if "nc" not in _nc_cache:
        _nc_cache["nc"] = _build_program()
    nc = _nc_cache["nc"]

    cache = {}
    in_maps = []
    for core in range(NCORES):
        m = {}
        for b in range(BPC):
            g = core * BPC + b
            key = (float(fl[g]), float(cam_dist[g]), g)
            wt_hi, wt_lo, q, p = _host_precompute(depth_t[g, 0], fl[g], cam_dist[g])
            m[f"wt_hi{b}"] = wt_hi
            m[f"wt_lo{b}"] = wt_lo
            m[f"q{b}"] = q
            m[f"p{b}"] = p
        in_maps.append(m)

    globals()["_last_in_maps"] = in_maps
    r = run_bass_kernel_spmd(nc, in_maps, list(range(NCORES)))

    out = np.empty((N, 1, RES, RES, RES), dtype=np.float32)
    for core in range(NCORES):
        for b in range(BPC):
            g = core * BPC + b
            od = r.results[core][f"outdev{b}"].reshape(RES, RES, RES)  # [j,k,i]
            out[g, 0] = od.transpose(2, 0, 1)
    return out
